# revision 24
# baseline (speedup 1.0000x reference)
"""Trainium2 Bass kernel for nn_AttentionModel (GRU scan over 2048 players
+ attention pooling heads), distributed over 8 NeuronCores.

Strategy: windowed Jacobi (Picard) iteration. Each core owns a 256-position
chunk of the player sequence, extended NS positions to the left. Every sweep
recomputes h_t = GRUcell(h_{t-1}^{prev sweep}, gi_t) for the whole window as
one big PE-friendly matmul; the fixed left boundary (frozen zeros) error
decays geometrically with distance, so after NS sweeps the owned chunk is
converged to ~gamma^NS (~0.69^NS). Zero inter-core communication.

Precision: bf16 matmuls for the first sweeps, float32r (full-rate 4-byte
matmul) for tail sweeps; f32 master state carries between sweeps.

kernel(**inputs) takes the FULL unsharded inputs and returns
(a_pol[3], d_pol[3], v[1]) as numpy arrays, matching reference().
"""

import math
from contextlib import ExitStack

import ml_dtypes
import numpy as np

import concourse.bass as bass
import concourse.tile as tile
from concourse import bacc
from concourse import mybir
from concourse.alu_op_type import AluOpType
from concourse.bass_utils import run_bass_kernel_spmd

F32 = mybir.dt.float32
F32R = mybir.dt.float32r
BF16 = mybir.dt.bfloat16
FP8 = mybir.dt.float8e4
AF = mybir.ActivationFunctionType
FP8_SCALE = 1024.0

P, OBS, H, G3 = 2048, 512, 2048, 6144
NCORES, CH = 8, 256
KT = H // 128          # 16 h-dim K tiles
MT = G3 // 128         # 48 gate-dim M tiles

# schedule: ns_fp8 fp8-DoubleRow sweeps, ns_bf16 bf16 sweeps, ns_tail f32r
DEFAULT_CFG = dict(ns_fp8=10, ns_bf16=5, ns_tail=5, res_f32r=3,
                   trace=False)


def _hcol(lp, k, t):
    return k * lp + t


def build_kernel(cfg):
    ns_fp8, ns_bf, ns_tail = cfg["ns_fp8"], cfg["ns_bf16"], cfg["ns_tail"]
    NS = ns_fp8 + ns_bf + ns_tail
    L = CH + NS            # window length
    assert L % 2 == 0, "fp32r matmul needs an even moving dim"
    LP = ((L + 1 + 15) // 16) * 16   # 16-aligned column stride per k-tile
                                     # (col 0 = frozen-zero boundary)

    def mode_of(s):
        return "fp8" if s < ns_fp8 else ("bf" if s < ns_fp8 + ns_bf else "f32r")

    MIR_DT = {"fp8": FP8, "bf": BF16, "f32r": F32R}
    MIR_PAD = {"fp8": [128, KT * LP * 4], "bf": [128, KT * LP * 2],
               "f32r": None}

    nc = bacc.Bacc()

    # ---- kernel I/O ----
    obs_t = nc.declare_dram_parameter("obs_t", [OBS, L], F32R, isOutput=False)
    # W streams pre-arranged per triple: [triple, 128p, (gate, ktile, 128c)]
    w_sbf = nc.declare_dram_parameter("w_sbf", [KT, 128, 3 * KT * 128], BF16,
                                      isOutput=False)
    wih_s = nc.declare_dram_parameter("wih_s", [MT, 128, 4 * 128], F32R,
                                      isOutput=False)
    bias_act = nc.declare_dram_parameter("bias_act", [128, MT], F32, isOutput=False)
    bias_hn = nc.declare_dram_parameter("bias_hn", [128, KT], F32, isOutput=False)
    ffw = nc.declare_dram_parameter("ffw", [128, KT * 2], F32, isOutput=False)
    zpatch = nc.declare_dram_parameter("zpatch", [128, NS], F32, isOutput=False)
    zbias = nc.declare_dram_parameter("zbias", [128, NS], F32, isOutput=False)
    if ns_tail:
        w_sf32 = nc.declare_dram_parameter("w_sf32", [KT, 128, 3 * KT * 128],
                                           F32R, isOutput=False)
    if ns_fp8:
        w_s8 = nc.declare_dram_parameter("w_s8", [KT, 128, 3 * KT * 128],
                                         FP8, isOutput=False)

    sa_o = nc.declare_dram_parameter("sa", [128, KT], F32, isOutput=True)
    sd_o = nc.declare_dram_parameter("sd", [128, KT], F32, isOutput=True)
    es_o = nc.declare_dram_parameter("es", [2, 1], F32, isOutput=True)
    hdbg_o = nc.declare_dram_parameter("hdbg", [128, KT * CH], F32,
                                       isOutput=True)



    with tile.TileContext(nc, trace_sim=bool(cfg.get("sim_trace"))) as tc, \
            ExitStack() as ctx:
        const = ctx.enter_context(tc.tile_pool(name="const", bufs=1))
        wres_p = ctx.enter_context(tc.tile_pool(name="wres", bufs=1))
        wstr_p = ctx.enter_context(tc.tile_pool(name="wstr", bufs=3))
        hpool = ctx.enter_context(tc.tile_pool(name="h", bufs=1))
        gip = ctx.enter_context(tc.tile_pool(name="gi", bufs=6))
        tmp = ctx.enter_context(tc.tile_pool(name="tmp", bufs=2))
        psum = ctx.enter_context(tc.tile_pool(name="psum", bufs=6, space="PSUM"))

        # ---- constants ----
        bias_sb = const.tile([128, MT], F32, tag="bias")
        nc.sync.dma_start(bias_sb[:], bias_act[:, :])
        bhn_sb = const.tile([128, KT], F32, tag="bhn")
        nc.sync.dma_start(bhn_sb[:], bias_hn[:, :])
        ffw_sb = const.tile([128, KT * 2], F32, tag="ffw")
        nc.sync.dma_start(ffw_sb[:], ffw[:, :])
        zp_sb = const.tile([128, NS], F32, tag="zp")
        nc.sync.dma_start(zp_sb[:], zpatch[:, :])
        zb_sb = const.tile([128, NS], F32, tag="zb")
        nc.sync.dma_start(zb_sb[:], zbias[:, :])
        obs_sb = const.tile([128, 4 * L], F32R, tag="obs")
        for ko in range(4):
            nc.sync.dma_start(obs_sb[:, ko * L:(ko + 1) * L],
                              obs_t[ko * 128:(ko + 1) * 128, :])

        # ---- resident f32r W part-tiles for the tail (first RESF of 48,
        # in consumption order f = i*3+part) ----
        RESF = cfg.get("res_f32r", 0)
        if RESF and ns_tail:
            wres = wres_p.tile([128, RESF * KT * 128], F32R, tag="wresf")
            for f in range(RESF):
                i, part = f // 3, f % 3
                nc.scalar.dma_start(
                    wres[:, f * KT * 128:(f + 1) * KT * 128],
                    w_sf32[i, :, part * KT * 128:(part + 1) * KT * 128])

        # ---- state buffers ----
        HA = hpool.tile([128, KT * LP], F32, tag="HA")
        HB = hpool.tile([128, KT * LP], F32, tag="HB")
        # mirror of the state in the matmul input dtype for the NEXT sweep
        # (bf16 during the bf16 phase, f32r during the tail), ping-ponged
        # through a 2-slot pool so both dtypes share the same SBUF.
        mirror_dt = MIR_DT[mode_of(0)]
        mir = hpool.tile([128, KT * LP], mirror_dt, tag="mir", bufs=2,
                         padded_shape=MIR_PAD[mode_of(0)])
        mfull = mir[:].bitcast(mybir.dt.uint8) if mirror_dt == FP8 else mir[:]
        nc.vector.memset(mfull, 0.0)
        for t in (HA, HB):
            nc.vector.memset(t[:], 0.0)

        # ---- GI = obs @ w_ih.T, resident in SBUF ([gate, seq] layout) ----
        gi_sb = gip.tile([128, MT * L], F32, tag="gi", bufs=1)

        def gi_slice(m):
            return gi_sb[:, m * L:(m + 1) * L]

        for m in range(MT):
            ps = psum.tile([128, L], F32, tag="gh")
            wtm = wstr_p.tile([128, 4 * 128], F32R, tag="wih", bufs=3)
            nc.sync.dma_start(wtm[:], wih_s[m])
            for ko in range(4):
                nc.tensor.matmul(ps[:], lhsT=wtm[:, ko * 128:(ko + 1) * 128],
                                 rhs=obs_sb[:, ko * L:(ko + 1) * L],
                                 start=(ko == 0), stop=(ko == 3))
            git = gi_slice(m)
            nc.vector.tensor_copy(git, ps[:])
            if KT <= m < 2 * KT:
                # z-gate patch: core 0 forces z=1 on its fake left-extension
                # positions so h stays exactly 0 there (zpatch=0, zbias=30);
                # other cores pass identity (zpatch=1, zbias=0).
                nc.vector.tensor_tensor(git[:, 0:NS], git[:, 0:NS], zp_sb[:],
                                        AluOpType.mult)
                nc.vector.tensor_tensor(git[:, 0:NS], git[:, 0:NS], zb_sb[:],
                                        AluOpType.add)

        # ---- sweeps ----
        inv8 = 1.0 / FP8_SCALE
        for s in range(NS):
            md = mode_of(s)
            Hin, Hout = (HA, HB) if s % 2 == 0 else (HB, HA)
            Bin = mir
            Bin3 = Bin[:].rearrange("p (k t) -> p k t", k=KT)
            nmd = None if s == NS - 1 else mode_of(s + 1)
            if nmd is not None:
                nxt_dt = MIR_DT[nmd]
                mir = hpool.tile([128, KT * LP], nxt_dt, tag="mir", bufs=2,
                                 padded_shape=MIR_PAD[nmd])
                # only the boundary col (t=0) of each k-tile needs zeroing;
                # data cols are overwritten by the h_new copies below
                bnd = mir[:].rearrange("p (k t) -> p k t", k=KT)[:, :, 0:1]
                if nxt_dt == FP8:
                    bnd = bnd.bitcast(mybir.dt.uint8)
                elif nxt_dt == F32R:
                    bnd = bnd.bitcast(F32)
                nc.vector.memset(bnd, 0.0)
                Bout = mir
            else:
                Bout = None
            for i in range(KT):
                pss = []
                for part in range(3):
                    wt = None
                    # one contiguous DMA per (triple, gate) W block, issued
                    # round-robin across engine queues
                    eng = (nc.sync, nc.sync, nc.scalar)[(i * 3 + part) % 3]
                    if md == "f32r" and RESF and (i * 3 + part) < RESF:
                        f = i * 3 + part
                        wt = None
                        wtap = wres[:, f * KT * 128:(f + 1) * KT * 128]
                    elif md == "f32r":
                        wt = wstr_p.tile([128, KT * 128], F32R, tag="wst",
                                         bufs=3)
                        eng.dma_start(
                            wt[:], w_sf32[i, :, part * KT * 128:
                                          (part + 1) * KT * 128])
                    elif md == "bf":
                        wt = wstr_p.tile([128, KT * 128], BF16, tag="wst",
                                         bufs=3,
                                         padded_shape=[128, KT * 128 * 2])
                        eng.dma_start(
                            wt[:], w_sbf[i, :, part * KT * 128:
                                         (part + 1) * KT * 128])
                    else:
                        wt = wstr_p.tile([128, KT * 128], FP8, tag="wst",
                                         bufs=3,
                                         padded_shape=[128, KT * 128 * 4])
                        eng.dma_start(
                            wt[:], w_s8[i, :, part * KT * 128:
                                        (part + 1) * KT * 128])

                    if wt is not None:
                        wtap = wt[:]
                    ps = psum.tile([128, L], F32, tag="gh")
                    if md == "fp8":
                        wt3 = wtap.rearrange("p (k c) -> p k c", k=KT)
                        for kp in range(KT // 2):
                            nc.tensor.matmul(
                                ps[:],
                                lhsT=wt3[:, 2 * kp:2 * kp + 2, :],
                                rhs=Bin3[:, 2 * kp:2 * kp + 2, 0:L],
                                start=(kp == 0), stop=(kp == KT // 2 - 1),
                                perf_mode=mybir.MatmulPerfMode.DoubleRow)
                    else:
                        for kt in range(KT):
                            rhs = Bin[:, _hcol(LP, kt, 0):_hcol(LP, kt, L)]
                            nc.tensor.matmul(
                                ps[:],
                                lhsT=wtap[:, kt * 128:(kt + 1) * 128],
                                rhs=rhs,
                                start=(kt == 0), stop=(kt == KT - 1))
                    pss.append(ps)

                gir = gi_slice(i)
                giz = gi_slice(KT + i)
                gin = gi_slice(2 * KT + i)

                gdt = BF16 if md == "fp8" else F32
                r = tmp.tile([128, L], gdt, tag="r", bufs=3,
                             padded_shape=[128, 2 * L] if gdt == BF16 else None)
                z = tmp.tile([128, L], gdt, tag="z", bufs=3,
                             padded_shape=[128, 2 * L] if gdt == BF16 else None)
                rhn = tmp.tile([128, L], gdt, tag="rhn", bufs=2,
                               padded_shape=[128, 2 * L] if gdt == BF16 else None)
                if md == "fp8":
                    # psum holds FP8_SCALE * gh — unscale while adding gi
                    nc.vector.scalar_tensor_tensor(
                        r[:], pss[0][:], inv8, gir,
                        AluOpType.mult, AluOpType.add)
                    nc.vector.scalar_tensor_tensor(
                        z[:], pss[1][:], inv8, giz,
                        AluOpType.mult, AluOpType.add)
                    ghn = tmp.tile([128, L], gdt, tag="ghn", bufs=2,
                                   padded_shape=[128, 2 * L])
                    nc.vector.tensor_scalar(ghn[:], pss[2][:], inv8,
                                            bhn_sb[:, i:i + 1],
                                            AluOpType.mult, AluOpType.add)
                else:
                    nc.vector.tensor_tensor(r[:], gir, pss[0][:],
                                            AluOpType.add)
                    nc.vector.tensor_tensor(z[:], giz, pss[1][:],
                                            AluOpType.add)
                nc.scalar.activation(r[:], r[:], AF.Sigmoid,
                                     bias=bias_sb[:, i:i + 1])
                nc.scalar.activation(z[:], z[:], AF.Sigmoid,
                                     bias=bias_sb[:, KT + i:KT + i + 1])
                # rhn = (ghn + b_hh_n) * r
                if md == "fp8":
                    nc.vector.tensor_tensor(rhn[:], ghn[:], r[:],
                                            AluOpType.mult)
                else:
                    nc.vector.scalar_tensor_tensor(rhn[:], pss[2][:],
                                                   bhn_sb[:, i:i + 1], r[:],
                                                   AluOpType.add,
                                                   AluOpType.mult)
                n = tmp.tile([128, L], gdt, tag="n", bufs=3,
                             padded_shape=[128, 2 * L] if gdt == BF16 else None)
                nc.vector.tensor_tensor(n[:], rhn[:], gin, AluOpType.add)
                nc.scalar.activation(n[:], n[:], AF.Tanh,
                                     bias=bias_sb[:, 2 * KT + i:2 * KT + i + 1])
                # h_new = (hp - n) * z + n   (fp8 sweeps: offload the two
                # intermediate ops to the otherwise-idle GpSimd engine)
                hp = Hin[:, _hcol(LP, i, 0):_hcol(LP, i, L)]
                d = tmp.tile([128, L], gdt, tag="d", bufs=2,
                             padded_shape=[128, 2 * L] if gdt == BF16 else None)
                deng = nc.gpsimd if md == "fp8" else nc.vector
                deng.tensor_tensor(d[:], hp, n[:], AluOpType.subtract)
                deng.tensor_tensor(d[:], d[:], z[:], AluOpType.mult)
                hnew = Hout[:, _hcol(LP, i, 1):_hcol(LP, i, L + 1)]
                nc.vector.tensor_tensor(hnew, d[:], n[:], AluOpType.add)
                if Bout is not None:
                    nc.vector.tensor_copy(
                        Bout[:, _hcol(LP, i, 1):_hcol(LP, i, L + 1)], hnew)

        Hfin = HB if NS % 2 == 1 else HA

        # ---- final: acts -> leaky -> exp -> weighted partial sums ----
        def chunk(k):  # own 256 columns of K-tile k (h values, not boundary)
            return Hfin[:, _hcol(LP, k, L + 1 - CH):_hcol(LP, k, L + 1)]

        nc.sync.dma_start(hdbg_o.rearrange("p (kt t) -> p kt t", kt=KT),
                          Hfin[:].rearrange("p (kt t) -> p kt t", kt=KT)
                          [:, :, L + 1 - CH:L + 1])

        psa = psum.tile([128, CH], F32, tag="acts", bufs=1)
        for kt in range(KT):
            nc.tensor.matmul(psa[0:2, :], lhsT=ffw_sb[:, kt * 2:kt * 2 + 2],
                             rhs=chunk(kt), start=(kt == 0), stop=(kt == KT - 1))
        # leaky relu: 0.505*x + 0.495*|x|
        ab = tmp.tile([128, CH], F32, tag="fin", bufs=3)
        nc.scalar.activation(ab[0:2, :], psa[0:2, :], AF.Abs)
        x5 = tmp.tile([128, CH], F32, tag="fin", bufs=3)
        nc.scalar.activation(x5[0:2, :], psa[0:2, :], AF.Copy, scale=0.505)
        ew = tmp.tile([128, CH], F32, tag="fin", bufs=3)
        nc.vector.scalar_tensor_tensor(ew[0:2, :], ab[0:2, :], 0.495, x5[0:2, :],
                                       AluOpType.mult, AluOpType.add)
        nc.scalar.activation(ew[0:2, :], ew[0:2, :], AF.Exp)
        esum = tmp.tile([128, 1], F32, tag="esum", bufs=1)
        nc.vector.reduce_sum(esum[0:2, :], ew[0:2, :], axis=mybir.AxisListType.X)
        nc.sync.dma_start(es_o[:, :], esum[0:2, :])

        ewd0 = tmp.tile([128, CH], F32, tag="fin", bufs=3)
        nc.sync.dma_start(ewd0[0:1, :], ew[1:2, :])
        ewa = tmp.tile([128, CH], F32, tag="ewb", bufs=2)
        nc.gpsimd.partition_broadcast(ewa[:], ew[0:1, :])
        ewd = tmp.tile([128, CH], F32, tag="ewb", bufs=2)
        nc.gpsimd.partition_broadcast(ewd[:], ewd0[0:1, :])

        sa_sb = tmp.tile([128, KT], F32, tag="sa", bufs=1)
        sd_sb = tmp.tile([128, KT], F32, tag="sd", bufs=1)
        for kt in range(KT):
            m = tmp.tile([128, CH], F32, tag="fmul", bufs=2)
            nc.vector.tensor_tensor(m[:], chunk(kt), ewa[:], AluOpType.mult)
            nc.vector.reduce_sum(sa_sb[:, kt:kt + 1], m[:],
                                 axis=mybir.AxisListType.X)
            m2 = tmp.tile([128, CH], F32, tag="fmul", bufs=2)
            nc.vector.tensor_tensor(m2[:], chunk(kt), ewd[:], AluOpType.mult)
            nc.vector.reduce_sum(sd_sb[:, kt:kt + 1], m2[:],
                                 axis=mybir.AxisListType.X)
        nc.sync.dma_start(sa_o[:, :], sa_sb[:])
        nc.sync.dma_start(sd_o[:, :], sd_sb[:])

    nc.finalize()
    return nc


# ---------------- host side ----------------

def _prep_inputs(inputs, cfg):
    NS = cfg["ns_fp8"] + cfg["ns_bf16"] + cfg["ns_tail"]
    L = CH + NS
    obs = np.asarray(inputs["obs"], np.float32)
    w_ih = np.asarray(inputs["w_ih"], np.float32)
    w_hh = np.asarray(inputs["w_hh"], np.float32)
    b_ih = np.asarray(inputs["b_ih"], np.float32)
    b_hh = np.asarray(inputs["b_hh"], np.float32)
    ff_w = np.asarray(inputs["ff_w"], np.float32)

    w_t32 = np.ascontiguousarray(w_hh.T)                      # [H, G3]
    # per-triple contiguous stream layout: [m, p, (g, kt, c)]
    wv = w_t32.reshape(KT, 128, 3, KT, 128).transpose(3, 1, 2, 0, 4)
    w_s32 = np.ascontiguousarray(wv.reshape(KT, 128, 3 * KT * 128))
    w_sbf = w_s32.astype(ml_dtypes.bfloat16)
    w_s8 = (w_s32 * FP8_SCALE).astype(ml_dtypes.float8_e4m3)
    wihv = w_ih.T.reshape(4, 128, MT, 128).transpose(2, 1, 0, 3)
    wih_s = np.ascontiguousarray(wihv.reshape(MT, 128, 4 * 128))

    # biases laid out per M-tile column: [128, MT]; r/z cols get b_ih+b_hh,
    # n cols get b_ih only (b_hh n-part applied inside the r*hn term).
    bsum = b_ih + b_hh
    bias_act = np.empty((128, MT), np.float32)
    for m in range(MT):
        src = bsum if m < 2 * KT else b_ih
        bias_act[:, m] = src[m * 128:(m + 1) * 128]
    bias_hn = np.empty((128, KT), np.float32)
    for k in range(KT):
        bias_hn[:, k] = b_hh[(2 * KT + k) * 128:(2 * KT + k + 1) * 128]
    ffw_h = np.empty((128, KT * 2), np.float32)
    for k in range(KT):
        ffw_h[:, 2 * k] = ff_w[0, k * 128:(k + 1) * 128]
        ffw_h[:, 2 * k + 1] = ff_w[1, k * 128:(k + 1) * 128]

    maps = []
    for c in range(NCORES):
        end = (c + 1) * CH
        w0 = end - L                       # window start (may be negative)
        obs_win = np.zeros((L, OBS), np.float32)
        lo = max(0, -w0)
        obs_win[lo:] = obs[w0 + lo:end]
        fake = lo                          # number of fake positions (core 0)
        zp = np.ones((128, NS), np.float32)
        zb = np.zeros((128, NS), np.float32)
        zp[:, :fake] = 0.0
        zb[:, :fake] = 30.0
        m = dict(obs_t=np.ascontiguousarray(obs_win.T), w_sbf=w_sbf,
                 wih_s=wih_s, bias_act=bias_act, bias_hn=bias_hn,
                 ffw=ffw_h, zpatch=zp, zbias=zb)
        if cfg["ns_tail"]:
            m["w_sf32"] = w_s32
        if cfg["ns_fp8"]:
            m["w_s8"] = w_s8
        maps.append(m)
    return maps


def _combine(results, inputs):
    ap_w = np.asarray(inputs["ap_w"], np.float64)
    ap_b = np.asarray(inputs["ap_b"], np.float64)
    dp_w = np.asarray(inputs["dp_w"], np.float64)
    dp_b = np.asarray(inputs["dp_b"], np.float64)
    v_w = np.asarray(inputs["v_w"], np.float64)
    v_b = np.asarray(inputs["v_b"], np.float64)

    sa = np.zeros(H, np.float64)
    sd = np.zeros(H, np.float64)
    ea = ed = 0.0
    for r in results:
        sa += r["sa"].astype(np.float64).T.ravel()
        sd += r["sd"].astype(np.float64).T.ravel()
        ea += float(r["es"][0, 0])
        ed += float(r["es"][1, 0])
    a_att = sa / ea
    d_att = sd / ed
    a_pol = (a_att @ ap_w.T + ap_b).astype(np.float32)
    d_pol = (d_att @ dp_w.T + dp_b).astype(np.float32)
    v = ((a_att + d_att) @ v_w.T + v_b).astype(np.float32)
    return a_pol, d_pol, v


_BUILT = {}


def run_cores(inputs, cfg=None):
    """Run the SPMD kernel, return (per-core raw results, combined outputs)."""
    cfg = dict(DEFAULT_CFG, **(cfg or {}))
    key = (cfg["ns_fp8"], cfg["ns_bf16"], cfg["ns_tail"])
    if key not in _BUILT:
        _BUILT[key] = build_kernel(cfg)
    nc = _BUILT[key]
    maps = _prep_inputs(inputs, cfg)
    res = run_bass_kernel_spmd(nc, maps, list(range(NCORES)),
                               trace=cfg.get("trace", False))
    outs = _combine(res.results, inputs)
    return res, outs


def kernel(**inputs):
    _, outs = run_cores(inputs)
    return outs


# revision 25
# speedup vs baseline: 1.1025x; 1.1025x over previous
"""Trainium2 Bass kernel for nn_AttentionModel (GRU scan over 2048 players
+ attention pooling heads), distributed over 8 NeuronCores.

Strategy: windowed Jacobi (Picard) iteration. Each core owns a 256-position
chunk of the player sequence, extended NS positions to the left. Every sweep
recomputes h_t = GRUcell(h_{t-1}^{prev sweep}, gi_t) for the whole window as
one big PE-friendly matmul; the fixed left boundary (frozen zeros) error
decays geometrically with distance, so after NS sweeps the owned chunk is
converged to ~gamma^NS (~0.69^NS). Zero inter-core communication.

Precision schedule: fp8(e4m3)+DoubleRow matmuls for the early sweeps,
bf16 for the middle, float32r (full-rate 4-byte matmul) for the tail;
an f32 master state carries between sweeps, with per-sweep low-precision
mirrors feeding the PE. Accuracy vs the f64 reference: ~1e-5 absolute on
outputs whose absmax is ~0.039 (rel ~2.7e-4).

kernel(**inputs) takes the FULL unsharded inputs and returns
(a_pol[3], d_pol[3], v[1]) as numpy arrays, matching reference().
"""

from contextlib import ExitStack

import ml_dtypes
import numpy as np

import concourse.tile as tile
from concourse import bacc
from concourse import mybir
from concourse.alu_op_type import AluOpType
from concourse.bass_utils import run_bass_kernel_spmd

F32 = mybir.dt.float32
F32R = mybir.dt.float32r
BF16 = mybir.dt.bfloat16
FP8 = mybir.dt.float8e4
AF = mybir.ActivationFunctionType
FP8_SCALE = 1024.0

P, OBS, H, G3 = 2048, 512, 2048, 6144
NCORES, CH = 8, 256
KT = H // 128          # 16 h-dim K tiles
MT = G3 // 128         # 48 gate-dim M tiles

# schedule: ns_fp8 fp8-DoubleRow sweeps, ns_bf16 bf16 sweeps, ns_tail f32r
DEFAULT_CFG = dict(ns_fp8=10, ns_bf16=5, ns_tail=5, res_f32r=3,
                   trace=False)


def _hcol(lp, k, t):
    return k * lp + t


def build_kernel(cfg):
    ns_fp8, ns_bf, ns_tail = cfg["ns_fp8"], cfg["ns_bf16"], cfg["ns_tail"]
    NS = ns_fp8 + ns_bf + ns_tail
    L = CH + NS            # window length
    assert L % 2 == 0, "fp32r matmul needs an even moving dim"
    LP = ((L + 1 + 15) // 16) * 16   # 16-aligned column stride per k-tile
                                     # (col 0 = frozen-zero boundary)

    def mode_of(s):
        return "fp8" if s < ns_fp8 else ("bf" if s < ns_fp8 + ns_bf else "f32r")

    MIR_DT = {"fp8": FP8, "bf": BF16, "f32r": F32R}
    MIR_PAD = {"fp8": [128, KT * LP * 4], "bf": [128, KT * LP * 2],
               "f32r": None}

    nc = bacc.Bacc()

    # ---- kernel I/O ----
    obs_t = nc.declare_dram_parameter("obs_t", [OBS, L], F32R, isOutput=False)
    # W streams pre-arranged per triple: [triple, 128p, (gate, ktile, 128c)]
    w_sbf = nc.declare_dram_parameter("w_sbf", [KT, 128, 3 * KT * 128], BF16,
                                      isOutput=False)
    wih_s = nc.declare_dram_parameter("wih_s", [MT, 128, 4 * 128], F32R,
                                      isOutput=False)
    bias_act = nc.declare_dram_parameter("bias_act", [128, MT], F32, isOutput=False)
    bias_hn = nc.declare_dram_parameter("bias_hn", [128, KT], F32, isOutput=False)
    ffw = nc.declare_dram_parameter("ffw", [128, KT * 2], F32, isOutput=False)
    zpatch = nc.declare_dram_parameter("zpatch", [128, NS], F32, isOutput=False)
    zbias = nc.declare_dram_parameter("zbias", [128, NS], F32, isOutput=False)
    if ns_tail:
        w_sf32 = nc.declare_dram_parameter("w_sf32", [KT, 128, 3 * KT * 128],
                                           F32R, isOutput=False)
    if ns_fp8:
        w_s8 = nc.declare_dram_parameter("w_s8", [KT, 128, 3 * KT * 128],
                                         FP8, isOutput=False)

    sa_o = nc.declare_dram_parameter("sa", [128, KT], F32, isOutput=True)
    sd_o = nc.declare_dram_parameter("sd", [128, KT], F32, isOutput=True)
    es_o = nc.declare_dram_parameter("es", [2, 1], F32, isOutput=True)
    hdbg_o = nc.declare_dram_parameter("hdbg", [128, KT * CH], F32,
                                       isOutput=True)



    with tile.TileContext(nc, trace_sim=bool(cfg.get("sim_trace"))) as tc, \
            ExitStack() as ctx:
        const = ctx.enter_context(tc.tile_pool(name="const", bufs=1))
        wres_p = ctx.enter_context(tc.tile_pool(name="wres", bufs=1))
        wstr_p = ctx.enter_context(tc.tile_pool(name="wstr", bufs=3))
        hpool = ctx.enter_context(tc.tile_pool(name="h", bufs=1))
        gip = ctx.enter_context(tc.tile_pool(name="gi", bufs=6))
        tmp = ctx.enter_context(tc.tile_pool(name="tmp", bufs=2))
        psum = ctx.enter_context(tc.tile_pool(name="psum", bufs=6, space="PSUM"))

        # ---- constants ----
        bias_sb = const.tile([128, MT], F32, tag="bias")
        nc.sync.dma_start(bias_sb[:], bias_act[:, :])
        bhn_sb = const.tile([128, KT], F32, tag="bhn")
        nc.sync.dma_start(bhn_sb[:], bias_hn[:, :])
        ffw_sb = const.tile([128, KT * 2], F32, tag="ffw")
        nc.sync.dma_start(ffw_sb[:], ffw[:, :])
        zp_sb = const.tile([128, NS], F32, tag="zp")
        nc.sync.dma_start(zp_sb[:], zpatch[:, :])
        zb_sb = const.tile([128, NS], F32, tag="zb")
        nc.sync.dma_start(zb_sb[:], zbias[:, :])
        obs_sb = const.tile([128, 4 * L], F32R, tag="obs")
        for ko in range(4):
            nc.sync.dma_start(obs_sb[:, ko * L:(ko + 1) * L],
                              obs_t[ko * 128:(ko + 1) * 128, :])

        # ---- resident f32r W part-tiles for the tail (first RESF of 48,
        # in consumption order f = i*3+part) ----
        RESF = cfg.get("res_f32r", 0)
        if RESF and ns_tail:
            wres = wres_p.tile([128, RESF * KT * 128], F32R, tag="wresf")
            for f in range(RESF):
                i, part = f // 3, f % 3
                nc.scalar.dma_start(
                    wres[:, f * KT * 128:(f + 1) * KT * 128],
                    w_sf32[i, :, part * KT * 128:(part + 1) * KT * 128])

        # ---- state buffers ----
        HA = hpool.tile([128, KT * LP], F32, tag="HA")
        HB = hpool.tile([128, KT * LP], F32, tag="HB")
        # mirror of the state in the matmul input dtype for the NEXT sweep
        # (bf16 during the bf16 phase, f32r during the tail), ping-ponged
        # through a 2-slot pool so both dtypes share the same SBUF.
        mirror_dt = MIR_DT[mode_of(0)]
        mir = hpool.tile([128, KT * LP], mirror_dt, tag="mir", bufs=2,
                         padded_shape=MIR_PAD[mode_of(0)])
        mfull = mir[:].bitcast(mybir.dt.uint8) if mirror_dt == FP8 else mir[:]
        nc.vector.memset(mfull, 0.0)
        for t in (HA, HB):
            nc.vector.memset(t[:], 0.0)

        # ---- GI = obs @ w_ih.T, resident in SBUF ([gate, seq] layout) ----
        gi_sb = gip.tile([128, MT * L], F32, tag="gi", bufs=1)

        def gi_slice(m):
            return gi_sb[:, m * L:(m + 1) * L]

        for m in range(MT):
            ps = psum.tile([128, L], F32, tag="gh")
            wtm = wstr_p.tile([128, 4 * 128], F32R, tag="wih", bufs=3)
            nc.sync.dma_start(wtm[:], wih_s[m])
            for ko in range(4):
                nc.tensor.matmul(ps[:], lhsT=wtm[:, ko * 128:(ko + 1) * 128],
                                 rhs=obs_sb[:, ko * L:(ko + 1) * L],
                                 start=(ko == 0), stop=(ko == 3))
            git = gi_slice(m)
            nc.vector.tensor_copy(git, ps[:])
            if KT <= m < 2 * KT:
                # z-gate patch: core 0 forces z=1 on its fake left-extension
                # positions so h stays exactly 0 there (zpatch=0, zbias=30);
                # other cores pass identity (zpatch=1, zbias=0).
                nc.vector.tensor_tensor(git[:, 0:NS], git[:, 0:NS], zp_sb[:],
                                        AluOpType.mult)
                nc.vector.tensor_tensor(git[:, 0:NS], git[:, 0:NS], zb_sb[:],
                                        AluOpType.add)

        # ---- sweeps ----
        inv8 = 1.0 / FP8_SCALE
        for s in range(NS):
            md = mode_of(s)
            Hin, Hout = (HA, HB) if s % 2 == 0 else (HB, HA)
            Bin = mir
            Bin3 = Bin[:].rearrange("p (k t) -> p k t", k=KT)
            nmd = None if s == NS - 1 else mode_of(s + 1)
            if nmd is not None:
                nxt_dt = MIR_DT[nmd]
                mir = hpool.tile([128, KT * LP], nxt_dt, tag="mir", bufs=2,
                                 padded_shape=MIR_PAD[nmd])
                # only the boundary col (t=0) of each k-tile needs zeroing;
                # data cols are overwritten by the h_new copies below
                bnd = mir[:].rearrange("p (k t) -> p k t", k=KT)[:, :, 0:1]
                if nxt_dt == FP8:
                    bnd = bnd.bitcast(mybir.dt.uint8)
                elif nxt_dt == F32R:
                    bnd = bnd.bitcast(F32)
                nc.vector.memset(bnd, 0.0)
                Bout = mir
            else:
                Bout = None
            for i in range(KT):
                pss = []
                for part in range(3):
                    wt = None
                    # one contiguous DMA per (triple, gate) W block, issued
                    # round-robin across engine queues
                    eng = (nc.sync, nc.sync, nc.scalar)[(i * 3 + part) % 3]
                    if md == "f32r" and RESF and (i * 3 + part) < RESF:
                        f = i * 3 + part
                        wt = None
                        wtap = wres[:, f * KT * 128:(f + 1) * KT * 128]
                    elif md == "f32r":
                        wt = wstr_p.tile([128, KT * 128], F32R, tag="wst",
                                         bufs=3)
                        eng.dma_start(
                            wt[:], w_sf32[i, :, part * KT * 128:
                                          (part + 1) * KT * 128])
                    elif md == "bf":
                        wt = wstr_p.tile([128, KT * 128], BF16, tag="wst",
                                         bufs=3,
                                         padded_shape=[128, KT * 128 * 2])
                        eng.dma_start(
                            wt[:], w_sbf[i, :, part * KT * 128:
                                         (part + 1) * KT * 128])
                    else:
                        wt = wstr_p.tile([128, KT * 128], FP8, tag="wst",
                                         bufs=3,
                                         padded_shape=[128, KT * 128 * 4])
                        eng.dma_start(
                            wt[:], w_s8[i, :, part * KT * 128:
                                        (part + 1) * KT * 128])

                    if wt is not None:
                        wtap = wt[:]
                    ps = psum.tile([128, L], F32, tag="gh")
                    if md == "fp8":
                        wt3 = wtap.rearrange("p (k c) -> p k c", k=KT)
                        for kp in range(KT // 2):
                            nc.tensor.matmul(
                                ps[:],
                                lhsT=wt3[:, 2 * kp:2 * kp + 2, :],
                                rhs=Bin3[:, 2 * kp:2 * kp + 2, 0:L],
                                start=(kp == 0), stop=(kp == KT // 2 - 1),
                                perf_mode=mybir.MatmulPerfMode.DoubleRow)
                    else:
                        for kt in range(KT):
                            rhs = Bin[:, _hcol(LP, kt, 0):_hcol(LP, kt, L)]
                            nc.tensor.matmul(
                                ps[:],
                                lhsT=wtap[:, kt * 128:(kt + 1) * 128],
                                rhs=rhs,
                                start=(kt == 0), stop=(kt == KT - 1))
                    pss.append(ps)

                gir = gi_slice(i)
                giz = gi_slice(KT + i)
                gin = gi_slice(2 * KT + i)

                gdt = BF16 if md == "fp8" else F32
                r = tmp.tile([128, L], gdt, tag="r", bufs=3,
                             padded_shape=[128, 2 * L] if gdt == BF16 else None)
                z = tmp.tile([128, L], gdt, tag="z", bufs=3,
                             padded_shape=[128, 2 * L] if gdt == BF16 else None)
                rhn = tmp.tile([128, L], gdt, tag="rhn", bufs=2,
                               padded_shape=[128, 2 * L] if gdt == BF16 else None)
                if md == "fp8":
                    # psum holds FP8_SCALE * gh — unscale while adding gi
                    nc.vector.scalar_tensor_tensor(
                        r[:], pss[0][:], inv8, gir,
                        AluOpType.mult, AluOpType.add)
                    nc.vector.scalar_tensor_tensor(
                        z[:], pss[1][:], inv8, giz,
                        AluOpType.mult, AluOpType.add)
                    ghn = tmp.tile([128, L], gdt, tag="ghn", bufs=2,
                                   padded_shape=[128, 2 * L])
                    nc.vector.tensor_scalar(ghn[:], pss[2][:], inv8,
                                            bhn_sb[:, i:i + 1],
                                            AluOpType.mult, AluOpType.add)
                else:
                    nc.vector.tensor_tensor(r[:], gir, pss[0][:],
                                            AluOpType.add)
                    nc.vector.tensor_tensor(z[:], giz, pss[1][:],
                                            AluOpType.add)
                nc.scalar.activation(r[:], r[:], AF.Sigmoid,
                                     bias=bias_sb[:, i:i + 1])
                nc.scalar.activation(z[:], z[:], AF.Sigmoid,
                                     bias=bias_sb[:, KT + i:KT + i + 1])
                # rhn = (ghn + b_hh_n) * r
                if md == "fp8":
                    nc.vector.tensor_tensor(rhn[:], ghn[:], r[:],
                                            AluOpType.mult)
                else:
                    nc.vector.scalar_tensor_tensor(rhn[:], pss[2][:],
                                                   bhn_sb[:, i:i + 1], r[:],
                                                   AluOpType.add,
                                                   AluOpType.mult)
                n = tmp.tile([128, L], gdt, tag="n", bufs=3,
                             padded_shape=[128, 2 * L] if gdt == BF16 else None)
                nc.vector.tensor_tensor(n[:], rhn[:], gin, AluOpType.add)
                nc.scalar.activation(n[:], n[:], AF.Tanh,
                                     bias=bias_sb[:, 2 * KT + i:2 * KT + i + 1])
                # h_new = (hp - n) * z + n   (fp8 sweeps: offload the two
                # intermediate ops to the otherwise-idle GpSimd engine)
                hp = Hin[:, _hcol(LP, i, 0):_hcol(LP, i, L)]
                d = tmp.tile([128, L], gdt, tag="d", bufs=2,
                             padded_shape=[128, 2 * L] if gdt == BF16 else None)
                deng = nc.gpsimd if md == "fp8" else nc.vector
                deng.tensor_tensor(d[:], hp, n[:], AluOpType.subtract)
                deng.tensor_tensor(d[:], d[:], z[:], AluOpType.mult)
                hnew = Hout[:, _hcol(LP, i, 1):_hcol(LP, i, L + 1)]
                nc.vector.tensor_tensor(hnew, d[:], n[:], AluOpType.add)
                if Bout is not None:
                    nc.vector.tensor_copy(
                        Bout[:, _hcol(LP, i, 1):_hcol(LP, i, L + 1)], hnew)

        Hfin = HB if NS % 2 == 1 else HA

        # ---- final: acts -> leaky -> exp -> weighted partial sums ----
        def chunk(k):  # own 256 columns of K-tile k (h values, not boundary)
            return Hfin[:, _hcol(LP, k, L + 1 - CH):_hcol(LP, k, L + 1)]

        nc.sync.dma_start(hdbg_o.rearrange("p (kt t) -> p kt t", kt=KT),
                          Hfin[:].rearrange("p (kt t) -> p kt t", kt=KT)
                          [:, :, L + 1 - CH:L + 1])

        psa = psum.tile([128, CH], F32, tag="acts", bufs=1)
        for kt in range(KT):
            nc.tensor.matmul(psa[0:2, :], lhsT=ffw_sb[:, kt * 2:kt * 2 + 2],
                             rhs=chunk(kt), start=(kt == 0), stop=(kt == KT - 1))
        # leaky relu: 0.505*x + 0.495*|x|
        ab = tmp.tile([128, CH], F32, tag="fin", bufs=3)
        nc.scalar.activation(ab[0:2, :], psa[0:2, :], AF.Abs)
        x5 = tmp.tile([128, CH], F32, tag="fin", bufs=3)
        nc.scalar.activation(x5[0:2, :], psa[0:2, :], AF.Copy, scale=0.505)
        ew = tmp.tile([128, CH], F32, tag="fin", bufs=3)
        nc.vector.scalar_tensor_tensor(ew[0:2, :], ab[0:2, :], 0.495, x5[0:2, :],
                                       AluOpType.mult, AluOpType.add)
        nc.scalar.activation(ew[0:2, :], ew[0:2, :], AF.Exp)
        esum = tmp.tile([128, 1], F32, tag="esum", bufs=1)
        nc.vector.reduce_sum(esum[0:2, :], ew[0:2, :], axis=mybir.AxisListType.X)
        nc.sync.dma_start(es_o[:, :], esum[0:2, :])

        ewd0 = tmp.tile([128, CH], F32, tag="fin", bufs=3)
        nc.sync.dma_start(ewd0[0:1, :], ew[1:2, :])
        ewa = tmp.tile([128, CH], F32, tag="ewb", bufs=2)
        nc.gpsimd.partition_broadcast(ewa[:], ew[0:1, :])
        ewd = tmp.tile([128, CH], F32, tag="ewb", bufs=2)
        nc.gpsimd.partition_broadcast(ewd[:], ewd0[0:1, :])

        sa_sb = tmp.tile([128, KT], F32, tag="sa", bufs=1)
        sd_sb = tmp.tile([128, KT], F32, tag="sd", bufs=1)
        for kt in range(KT):
            m = tmp.tile([128, CH], F32, tag="fmul", bufs=2)
            nc.vector.tensor_tensor(m[:], chunk(kt), ewa[:], AluOpType.mult)
            nc.vector.reduce_sum(sa_sb[:, kt:kt + 1], m[:],
                                 axis=mybir.AxisListType.X)
            m2 = tmp.tile([128, CH], F32, tag="fmul", bufs=2)
            nc.vector.tensor_tensor(m2[:], chunk(kt), ewd[:], AluOpType.mult)
            nc.vector.reduce_sum(sd_sb[:, kt:kt + 1], m2[:],
                                 axis=mybir.AxisListType.X)
        nc.sync.dma_start(sa_o[:, :], sa_sb[:])
        nc.sync.dma_start(sd_o[:, :], sd_sb[:])

    nc.finalize()
    return nc


# ---------------- host side ----------------

def _prep_inputs(inputs, cfg):
    NS = cfg["ns_fp8"] + cfg["ns_bf16"] + cfg["ns_tail"]
    L = CH + NS
    obs = np.asarray(inputs["obs"], np.float32)
    w_ih = np.asarray(inputs["w_ih"], np.float32)
    w_hh = np.asarray(inputs["w_hh"], np.float32)
    b_ih = np.asarray(inputs["b_ih"], np.float32)
    b_hh = np.asarray(inputs["b_hh"], np.float32)
    ff_w = np.asarray(inputs["ff_w"], np.float32)

    w_t32 = np.ascontiguousarray(w_hh.T)                      # [H, G3]
    # per-triple contiguous stream layout: [m, p, (g, kt, c)]
    wv = w_t32.reshape(KT, 128, 3, KT, 128).transpose(3, 1, 2, 0, 4)
    w_s32 = np.ascontiguousarray(wv.reshape(KT, 128, 3 * KT * 128))
    w_sbf = w_s32.astype(ml_dtypes.bfloat16)
    w_s8 = (w_s32 * FP8_SCALE).astype(ml_dtypes.float8_e4m3)
    wihv = w_ih.T.reshape(4, 128, MT, 128).transpose(2, 1, 0, 3)
    wih_s = np.ascontiguousarray(wihv.reshape(MT, 128, 4 * 128))

    # biases laid out per M-tile column: [128, MT]; r/z cols get b_ih+b_hh,
    # n cols get b_ih only (b_hh n-part applied inside the r*hn term).
    bsum = b_ih + b_hh
    bias_act = np.empty((128, MT), np.float32)
    for m in range(MT):
        src = bsum if m < 2 * KT else b_ih
        bias_act[:, m] = src[m * 128:(m + 1) * 128]
    bias_hn = np.empty((128, KT), np.float32)
    for k in range(KT):
        bias_hn[:, k] = b_hh[(2 * KT + k) * 128:(2 * KT + k + 1) * 128]
    ffw_h = np.empty((128, KT * 2), np.float32)
    for k in range(KT):
        ffw_h[:, 2 * k] = ff_w[0, k * 128:(k + 1) * 128]
        ffw_h[:, 2 * k + 1] = ff_w[1, k * 128:(k + 1) * 128]

    maps = []
    for c in range(NCORES):
        end = (c + 1) * CH
        w0 = end - L                       # window start (may be negative)
        obs_win = np.zeros((L, OBS), np.float32)
        lo = max(0, -w0)
        obs_win[lo:] = obs[w0 + lo:end]
        fake = lo                          # number of fake positions (core 0)
        zp = np.ones((128, NS), np.float32)
        zb = np.zeros((128, NS), np.float32)
        zp[:, :fake] = 0.0
        zb[:, :fake] = 30.0
        m = dict(obs_t=np.ascontiguousarray(obs_win.T), w_sbf=w_sbf,
                 wih_s=wih_s, bias_act=bias_act, bias_hn=bias_hn,
                 ffw=ffw_h, zpatch=zp, zbias=zb)
        if cfg["ns_tail"]:
            m["w_sf32"] = w_s32
        if cfg["ns_fp8"]:
            m["w_s8"] = w_s8
        maps.append(m)
    return maps


def _combine(results, inputs):
    ap_w = np.asarray(inputs["ap_w"], np.float64)
    ap_b = np.asarray(inputs["ap_b"], np.float64)
    dp_w = np.asarray(inputs["dp_w"], np.float64)
    dp_b = np.asarray(inputs["dp_b"], np.float64)
    v_w = np.asarray(inputs["v_w"], np.float64)
    v_b = np.asarray(inputs["v_b"], np.float64)

    sa = np.zeros(H, np.float64)
    sd = np.zeros(H, np.float64)
    ea = ed = 0.0
    for r in results:
        sa += r["sa"].astype(np.float64).T.ravel()
        sd += r["sd"].astype(np.float64).T.ravel()
        ea += float(r["es"][0, 0])
        ed += float(r["es"][1, 0])
    a_att = sa / ea
    d_att = sd / ed
    a_pol = (a_att @ ap_w.T + ap_b).astype(np.float32)
    d_pol = (d_att @ dp_w.T + dp_b).astype(np.float32)
    v = ((a_att + d_att) @ v_w.T + v_b).astype(np.float32)
    return a_pol, d_pol, v


_BUILT = {}


def run_cores(inputs, cfg=None):
    """Run the SPMD kernel, return (per-core raw results, combined outputs)."""
    cfg = dict(DEFAULT_CFG, **(cfg or {}))
    key = (cfg["ns_fp8"], cfg["ns_bf16"], cfg["ns_tail"])
    if key not in _BUILT:
        _BUILT[key] = build_kernel(cfg)
    nc = _BUILT[key]
    maps = _prep_inputs(inputs, cfg)
    res = run_bass_kernel_spmd(nc, maps, list(range(NCORES)),
                               trace=cfg.get("trace", False))
    outs = _combine(res.results, inputs)
    return res, outs


def kernel(**inputs):
    _, outs = run_cores(inputs)
    return outs
